# revision 64
# baseline (speedup 1.0000x reference)
"""Trainium2 Bass kernel for a single-layer attention module (RMSNorm + QKV +
RoPE + causal attention over a KV cache + output projection), tensor-parallel
over 8 NeuronCores (4 heads each), per-head AllGather of attention outputs,
and per-core output-column blocks of the final projection.

Merged-pipeline version: QKV projections of head h+1 and output-projection
matmuls are interleaved into attention head h's PE stream (hiding exp latency
and the collectives), softmax denominators are accumulated on the vector
engine (pair chains) instead of ones-matmuls, RMSNorm statistics are
accumulated on the vector engine, and score/PV matmuls use exact causal
widths.

Self-contained: takes FULL inputs, returns the FULL [1024, 4096] f32 output.
"""

import sys

sys.path.insert(0, "/opt/trn_rl_repo")

from collections import deque
from itertools import islice

import numpy as np
import ml_dtypes

import concourse.bass as bass  # noqa: F401
import concourse.bacc as bacc
import concourse.tile as tile
from concourse import mybir
from concourse import bass_utils

BF16 = ml_dtypes.bfloat16
F32 = np.float32

N_CORES = 8
D, H, HD, S, C = 4096, 32, 128, 1024, 2048
T = C + S          # 3072 total keys
HL = H // N_CORES  # 4 heads per core
OC = HL * HD       # 512 local attention features per core
NDK = D // 128     # 32 contraction tiles over D
NTC = C // 128     # 16 cache t-tiles
NTN = S // 128     # 8 new-key t-tiles
EPS = 1e-6
THETA = 10000.0

bf = mybir.dt.bfloat16
f32 = mybir.dt.float32


def _build_nc():
    nc = bacc.Bacc("TRN2", target_bir_lowering=False, debug=False,
                   num_devices=N_CORES)

    # ---- DRAM I/O ----
    xs_t = nc.dram_tensor("xs_t", [16, 128, 2 * S], bf, kind="ExternalInput")
    wq_col = nc.dram_tensor("wq_col", [HL, 128, NDK * 128], bf, kind="ExternalInput")
    wk_col = nc.dram_tensor("wk_col", [HL, 128, NDK * 128], bf, kind="ExternalInput")
    wv_col = nc.dram_tensor("wv_col", [HL, 128, NDK * 128], bf, kind="ExternalInput")
    wo_blk = nc.dram_tensor("wo_blk", [HL, 128, 8 * OC], bf, kind="ExternalInput")
    ckt = nc.dram_tensor("ckt", [HL, 128, C], bf, kind="ExternalInput")
    cvr = nc.dram_tensor("cvr", [HL, 128, C], bf, kind="ExternalInput")
    cosT = nc.dram_tensor("cosT", [128, S], bf, kind="ExternalInput")
    sinT = nc.dram_tensor("sinT", [128, S], bf, kind="ExternalInput")
    triW = nc.dram_tensor("triW", [128, 128], bf, kind="ExternalInput")
    ones_d = nc.dram_tensor("ones_d", [128, 128], bf, kind="ExternalInput")
    id_d = nc.dram_tensor("id_d", [128, 128], bf, kind="ExternalInput")
    rotP_d = nc.dram_tensor("rotP_d", [128, 128], bf, kind="ExternalInput")
    # y stored transposed ([outcol, s]); host transposes back
    y = nc.dram_tensor("y", [OC, S], f32, kind="ExternalOutput")
    import os
    _dbg = os.environ.get("KBG_DEBUG") == "1"
    if _dbg:
        rsq_dbg = nc.dram_tensor("rsq_dbg", [128, S], f32, kind="ExternalOutput")
        qr0_dbg = nc.dram_tensor("qr0_dbg", [128, S], bf, kind="ExternalOutput")
        kr0_dbg = nc.dram_tensor("kr0_dbg", [128, S], bf, kind="ExternalOutput")
        vs0_dbg = nc.dram_tensor("vs0_dbg", [128, S], bf, kind="ExternalOutput")
        hh2v_dbg = nc.dram_tensor("hh2v_dbg", [128, S], bf, kind="ExternalOutput")
        vs1_dbg = nc.dram_tensor("vs1_dbg", [128, S], bf, kind="ExternalOutput")
        hh2v1_dbg = nc.dram_tensor("hh2v1_dbg", [128, S], bf, kind="ExternalOutput")
        q1_dbg = nc.dram_tensor("q1_dbg", [128, S], bf, kind="ExternalOutput")
        vs1e_dbg = nc.dram_tensor("vs1e_dbg", [128, S], bf, kind="ExternalOutput")
        agsb_dbg = [nc.dram_tensor(f"agsb{i}_dbg", [128, 8 * S], bf,
                                   kind="ExternalOutput") for i in range(3)]
        at0_dbg = nc.dram_tensor("at0_dbg", [128, S], bf, kind="ExternalOutput")

    with tile.TileContext(nc) as tc:
        with (
            tc.tile_pool(name="const", bufs=1) as cpool,
            tc.tile_pool(name="qk", bufs=2) as qkpool,
            tc.tile_pool(name="att", bufs=2) as apool,
            tc.tile_pool(name="kv", bufs=2) as kvpool,
            tc.tile_pool(name="exp", bufs=6) as epool,
            tc.tile_pool(name="accp", bufs=5) as accpool,
            tc.tile_pool(name="rec", bufs=1) as recpool,
            tc.tile_pool(name="wo", bufs=4) as wopool,
            tc.tile_pool(name="dram", bufs=1, space="DRAM") as dpool,
            tc.tile_pool(name="psp", bufs=2, space="PSUM") as psp_pool,
            tc.tile_pool(name="psS", bufs=2, space="PSUM") as psS,
            tc.tile_pool(name="psO", bufs=2, space="PSUM") as psO,
        ):
            # ---- constants (tiles here; DMAs issued mid-stream in the
            # pre-region so they don't delay the first matmul's inputs) ----
            ones_t = cpool.tile([128, 128], bf, name="ones_t")
            id_t = cpool.tile([128, 128], bf, name="id_t")
            tri_t = cpool.tile([128, 128], bf, name="tri_t")
            rotP_t = cpool.tile([128, 128], bf, name="rotP_t")

            # per-head tile handles (ring depth 2: head h live + h+1 building)
            qr = [None] * HL
            kr = [None] * HL
            vsb = [None] * HL
            cks = [None] * HL
            cvs = [None] * HL
            wos = [None] * HL
            # collective dram buffers
            ag_in = [dpool.tile([128, S], bf, name=f"ag_in{h}") for h in range(3)]
            ag_out = [dpool.tile([N_CORES * 128, S], bf, name=f"ag_out{h}",
                                 addr_space="Shared") for h in range(3)]
            ag_in3 = [dpool.tile([128, 512], bf, name=f"ag_in3_{s}")
                      for s in range(2)]
            ag_out3 = [dpool.tile([N_CORES * 128, 512], bf, name=f"ag_out3_{s}",
                                  addr_space="Shared") for s in range(2)]

            box = {}       # rsq tile, transpose pool, transpose tile-name
            _DONE = object()   # drain sentinel (bare yields return None!)

            def groups_for(sc):
                # (kind, idx, off0): cache tiles full width, then new-key tiles
                # at exact causal width
                gs = [("c", ti, 0) for ti in range(NTC)]
                for tn in range(NTN):
                    if tn * 128 < (sc + 1) * 512:
                        gs.append(("n", tn, max(0, tn * 128 - sc * 512)))
                return gs

            def att_head(h, filler, psS, psO, last):
                """Emit attention for head h, draining filler between groups."""
                attnT = apool.tile([128, S], bf, name="attnT")
                box["attnT_last"] = attnT

                def drain(k):
                    for _ in range(k):
                        if next(filler, _DONE) is _DONE:
                            break

                # next head's weight DMAs first (needed ~20us before the
                # caches), then caches for head h+1
                if h + 1 < HL:
                    box["kick_qkv"](h + 1)
                # head 0: consume the whole gen0 tail (v matmuls, deferred
                # rsq/ropes, transposes) BEFORE any score matmul is emitted —
                # score matmuls must read the rope'd qr/kr, and the PE covers
                # the norm-chain latency with the v matmuls meanwhile
                drain(74 if h == 0 else 2)
                if h + 1 < HL:
                    cks[h + 1] = kvpool.tile([128, C], bf, name="ck_sb")
                    nc.sync.dma_start(cks[h + 1][:], ckt[h + 1])
                    cvs[h + 1] = kvpool.tile([128, C], bf, name="cv_sb")
                    nc.sync.dma_start(cvs[h + 1][:], cvr[h + 1])
                # wo weights are first needed at ATT_3's out-projection filler
                # (~300us in); stagger their 4MB well after the early phase
                if h in (1, 2):
                    for wi in (2 * h - 2, 2 * h - 1):
                        wos[wi] = wopool.tile([128, 8 * OC], bf, name="wo_sb")
                        nc.sync.dma_start(wos[wi][:], wo_blk[wi])
                for sc in range(2):
                    # pacing: light during DMA-bound att0, heavy later; must
                    # still emit vsb[h] transposes before the first n-retire
                    if last:
                        drain_k = 1   # keep ~146 outproj MMs for the AG3 wait
                    else:
                        drain_k = (4, 5, 5)[h]
                    gs = groups_for(sc)
                    oc_t = psO.tile([128, 512], f32, name="oc")
                    partials = [None, None]
                    stash = [None, None]
                    nnew = [0]

                    def retire(g, e):
                        kind, idx, off0 = g
                        lv = (cvs[h][:, idx * 128:(idx + 1) * 128]
                              if kind == "c" else
                              vsb[h][:, idx * 128:(idx + 1) * 128])
                        nc.tensor.matmul(
                            oc_t[:, off0:512], lv, e[:, off0:512],
                            start=(g == gs[0]), stop=(g == gs[-1]))
                        # softmax denominator accumulation on DVE
                        if kind == "c":
                            j = idx % 2
                            if partials[j] is None:
                                if stash[j] is None:
                                    stash[j] = e
                                else:
                                    p = accpool.tile([128, 512], bf, name="accp")
                                    nc.vector.tensor_add(p[:], stash[j][:], e[:])
                                    partials[j] = p
                                    stash[j] = None
                            else:
                                nc.vector.tensor_add(
                                    partials[j][:], partials[j][:], e[:])
                        else:
                            j = nnew[0] % 2
                            nnew[0] += 1
                            nc.vector.tensor_add(
                                partials[j][:, off0:512],
                                partials[j][:, off0:512], e[:, off0:512])

                    pend = deque()
                    for g in gs:
                        kind, idx, off0 = g
                        lk = (cks[h][:, idx * 128:(idx + 1) * 128]
                              if kind == "c" else
                              kr[h][:, idx * 128:(idx + 1) * 128])
                        ps = psS.tile([128, 512], f32, name="ps")
                        nc.tensor.matmul(
                            ps[:, off0:512], lk,
                            qr[h][:, sc * 512 + off0:(sc + 1) * 512],
                            start=True, stop=True)
                        e = epool.tile([128, 512], bf, name="e")
                        nc.scalar.activation(
                            e[:, off0:512], ps[:, off0:512],
                            mybir.ActivationFunctionType.Exp)
                        if kind == "n" and idx * 128 >= sc * 512:
                            nc.vector.tensor_mul(
                                e[:, off0:off0 + 128], e[:, off0:off0 + 128],
                                tri_t[:])
                        drain(drain_k)
                        if len(pend) >= 2:
                            retire(*pend.popleft())
                        pend.append((g, e))
                    while pend:
                        retire(*pend.popleft())

                    # den = colsum(partial0 + partial1): DVE pair-add, 1 ones-MM
                    pair = accpool.tile([128, 512], bf, name="accp")
                    nc.vector.tensor_add(pair[:], partials[0][:], partials[1][:])
                    den_ps = psS.tile([128, 512], f32, name="ps")
                    nc.tensor.matmul(den_ps[:], ones_t[:], pair[:],
                                     start=True, stop=True)
                    rec = recpool.tile([128, 512], f32, name="rec")
                    nc.vector.reciprocal_approx_fast(rec[:], den_ps[:])
                    nc.vector.tensor_mul(
                        attnT[:, sc * 512:(sc + 1) * 512], oc_t[:], rec[:])
                    if not last:
                        nc.gpsimd.dma_start(
                            ag_in[h][:, sc * 512:(sc + 1) * 512],
                            attnT[:, sc * 512:(sc + 1) * 512])
                    else:
                        nc.gpsimd.dma_start(
                            ag_in3[sc][:], attnT[:, sc * 512:(sc + 1) * 512])
                        nc.gpsimd.collective_compute(
                            "AllGather", mybir.AluOpType.bypass,
                            replica_groups=[list(range(N_CORES))],
                            ins=[ag_in3[sc][:]], outs=[ag_out3[sc][:]])
                        if "ag3_gather" in box:
                            box["ag3_gather"](sc)
                if not last:
                    nc.gpsimd.collective_compute(
                        "AllGather", mybir.AluOpType.bypass,
                        replica_groups=[list(range(N_CORES))],
                        ins=[ag_in[h][:]], outs=[ag_out[h][:]])

            # =============== pre-region: xs + QKV head 0 (+ norm) ===============
            with (
                tc.tile_pool(name="xs", bufs=16) as xpool,
                tc.tile_pool(name="wcol", bufs=4) as wpool,
                tc.tile_pool(name="hh", bufs=2) as hpool,
                tc.tile_pool(name="rope", bufs=1) as rpool,
                tc.tile_pool(name="nrm", bufs=1) as npool,
                tc.tile_pool(name="sq", bufs=3) as sqpool,
            ):
                # DMA issue order tracks PE need order. First pieces are small
                # so the first matmul can start ~8us in: xs dk0, wq dks 0-7,
                # wk dks 0-7, then the rest of each, then streaming chunks.
                xs_ch = []
                xc = xpool.tile([128, 2 * S], bf, name="xs_ch")
                nc.scalar.dma_start(xc[:, 0:S], xs_t[0][:, 0:S])
                xs_ch.append(xc)
                wcol0 = wpool.tile([128, NDK * 128], bf, name="wcol")
                nc.gpsimd.dma_start(wcol0[:, 0:1024], wq_col[0][:, 0:1024])
                wcol0k = wpool.tile([128, NDK * 128], bf, name="wcol")
                nc.sync.dma_start(wcol0k[:, 0:1024], wk_col[0][:, 0:1024])
                nc.sync.dma_start(xc[:, S:2 * S], xs_t[0][:, S:2 * S])
                nc.sync.dma_start(wcol0[:, 1024:NDK * 128],
                                  wq_col[0][:, 1024:NDK * 128])
                nc.sync.dma_start(wcol0k[:, 1024:NDK * 128],
                                  wk_col[0][:, 1024:NDK * 128])
                cos_t = cpool.tile([128, S], bf, name="cos_t")
                sin_t = cpool.tile([128, S], bf, name="sin_t")
                for g in range(1, 16):
                    xc = xpool.tile([128, 2 * S], bf, name="xs_ch")
                    nc.sync.dma_start(xc[:], xs_t[g])
                    xs_ch.append(xc)
                    if g == 8:
                        # constants needed from ~mid q/k projection onward
                        nc.sync.dma_start(cos_t[:], cosT[:])
                        nc.sync.dma_start(sin_t[:], sinT[:])
                        nc.sync.dma_start(ones_t[:], ones_d[:])
                        nc.sync.dma_start(id_t[:], id_d[:])
                        nc.sync.dma_start(tri_t[:], triW[:])
                        nc.sync.dma_start(rotP_t[:], rotP_d[:])
                wcol0v = wpool.tile([128, NDK * 128], bf, name="wcol")
                nc.sync.dma_start(wcol0v[:, 0:1024], wv_col[0][:, 0:1024])
                nc.sync.dma_start(wcol0v[:, 1024:NDK * 128],
                                  wv_col[0][:, 1024:NDK * 128])

                def xs_v(dk):  # [128, S] view of raw xs^T d-tile dk
                    return xs_ch[dk // 2][:, (dk % 2) * S:(dk % 2 + 1) * S]

                # head-0 caches early (needed right at ATT_0)
                cks[0] = kvpool.tile([128, C], bf, name="ck_sb")
                nc.sync.dma_start(cks[0][:, 0:1024], ckt[0][:, 0:1024])
                nc.sync.dma_start(cks[0][:, 1024:C], ckt[0][:, 1024:C])
                cvs[0] = kvpool.tile([128, C], bf, name="cv_sb")
                nc.sync.dma_start(cvs[0][:, 0:1024], cvr[0][:, 0:1024])
                nc.sync.dma_start(cvs[0][:, 1024:C], cvr[0][:, 1024:C])

                def rope(dst, src):
                    # dst = src * cos2 + rot(src) * sin2 (half-swap via DMA;
                    # hidden for heads 1-3 where attention overlaps it)
                    rot = rpool.tile([128, S], bf, name="rot")
                    nc.gpsimd.dma_start(rot[0:64, :], src[64:128, :])
                    nc.gpsimd.dma_start(rot[64:128, :], src[0:64, :])
                    ta = rpool.tile([128, S], bf, name="ta")
                    nc.vector.tensor_mul(ta[:], src[:], cos_t[:])
                    tb = rpool.tile([128, S], bf, name="tb")
                    nc.vector.tensor_mul(tb[:], rot[:], sin_t[:])
                    nc.vector.tensor_add(dst[:], ta[:], tb[:])

                def rope_pe(dst, src):
                    # half-swap on the PE (permutation matmul into psS halves)
                    # — keeps head 0's rope off the congested early DMA path
                    ta = rpool.tile([128, S], bf, name="ta")
                    nc.vector.tensor_mul(ta[:], src[:], cos_t[:])
                    for half in range(2):
                        sl = slice(half * 512, (half + 1) * 512)
                        rot_ps = psS.tile([128, 512], f32, name="ps")
                        nc.tensor.matmul(rot_ps[:], rotP_t[:], src[:, sl],
                                         start=True, stop=True)
                        tb = rpool.tile([128, 512], bf, name="tb")
                        nc.vector.tensor_mul(tb[:], rot_ps[:], sin_t[:, sl])
                        nc.vector.tensor_add(dst[:, sl], ta[:, sl], tb[:])

                wcs_of = {}

                def kick_qkv_dmas(h):
                    # allocate head-h QKV tiles + queue the 3 weight DMAs
                    qr[h] = qkpool.tile([128, S], bf, name="qr")
                    kr[h] = qkpool.tile([128, S], bf, name="kr")
                    vsb[h] = qkpool.tile([128, S], bf, name="vsb")
                    wcs = []
                    for wsrc in (wq_col, wk_col, wv_col):
                        wc = wpool.tile([128, NDK * 128], bf, name="wcol")
                        nc.sync.dma_start(wc[:], wsrc[h])
                        wcs.append(wc)
                    wcs_of[h] = wcs

                box["kick_qkv"] = kick_qkv_dmas

                def emit_qkv_head(h):
                    """Generator: emits QKV for head h, yields after each PE op."""
                    if h == 0:
                        qr[h] = qkpool.tile([128, S], bf, name="qr")
                        kr[h] = qkpool.tile([128, S], bf, name="kr")
                        vsb[h] = qkpool.tile([128, S], bf, name="vsb")
                        # q and k interleaved per dk: 8 matmuls per xs chunk
                        # keeps the PE ahead of the streaming xs DMA
                        psq = psp_pool.tile([128, S], f32, name="psp")
                        psk = psp_pool.tile([128, S], f32, name="psp")
                        for dk in range(NDK):
                            box["norm_dk"](dk)
                            for wc, pst in ((wcol0, psq), (wcol0k, psk)):
                                for scc in range(2):
                                    nc.tensor.matmul(
                                        pst[:, scc * 512:(scc + 1) * 512],
                                        wc[:, dk * 128:(dk + 1) * 128],
                                        xs_v(dk)[:, scc * 512:(scc + 1) * 512],
                                        start=(dk == 0), stop=(dk == NDK - 1))
                                    yield
                        # raw q/k into qr/kr (no rsq dep) frees the PSUM
                        # banks so the v matmuls start immediately; the
                        # normalize+rope chain runs after the v loop, hidden
                        # behind its ~17us of PE work
                        nc.scalar.copy(qr[0][:], psq[:])
                        nc.scalar.copy(kr[0][:], psk[:])
                        todo = (("v", wcol0v),)
                    else:
                        wcs = wcs_of[h]
                        todo = (("q", wcs[0]), ("k", wcs[1]), ("v", wcs[2]))
                    for which, wc in todo:
                        psp = psp_pool.tile([128, S], f32, name="psp")
                        for dk in range(NDK):
                            for scc in range(2):
                                nc.tensor.matmul(
                                    psp[:, scc * 512:(scc + 1) * 512],
                                    wc[:, dk * 128:(dk + 1) * 128],
                                    xs_v(dk)[:, scc * 512:(scc + 1) * 512],
                                    start=(dk == 0), stop=(dk == NDK - 1))
                                yield
                        if which != "v":
                            hh2 = hpool.tile([128, S], bf, name="hh2", bufs=2)
                            nc.vector.tensor_mul(hh2[:], psp[:], box["rsq"][:])
                            rope(qr[h][:] if which == "q" else kr[h][:], hh2)
                            if _dbg and h == 1 and which == "q":
                                nc.sync.dma_start(q1_dbg[:], qr[1][:])
                        else:
                            if h == 0:
                                # deferred: rsq, head-0 ropes (PE rot), then
                                # the shared v normalize+transpose path
                                box["rsq_emit"]()
                                hh2q = hpool.tile([128, S], bf, name="hh2",
                                                  bufs=2)
                                nc.vector.tensor_mul(hh2q[:], qr[0][:],
                                                     box["rsq"][:])
                                rope_pe(qr[0][:], hh2q)
                                hh2k = hpool.tile([128, S], bf, name="hh2",
                                                  bufs=2)
                                nc.vector.tensor_mul(hh2k[:], kr[0][:],
                                                     box["rsq"][:])
                                rope_pe(kr[0][:], hh2k)
                                yield
                            hh2 = hpool.tile([128, S], bf, name="hh2", bufs=2)
                            nc.vector.tensor_mul(hh2[:], psp[:],
                                                 box["rsq"][:])
                            if _dbg and h == 0:
                                nc.sync.dma_start(hh2v_dbg[:], hh2[:])
                            if _dbg and h == 1:
                                nc.sync.dma_start(hh2v1_dbg[:], hh2[:])
                            psT, ptr_name = box["psT"]
                            for tj in range(NTN):
                                ptr = psT.tile([128, 128], bf, name=ptr_name)
                                nc.tensor.transpose(
                                    ptr[:], hh2[:, tj * 128:(tj + 1) * 128],
                                    id_t[:])
                                nc.scalar.copy(
                                    vsb[h][:, tj * 128:(tj + 1) * 128], ptr[:])
                                yield
                            if _dbg and h == 1:
                                nc.sync.dma_start(vs1e_dbg[:], vsb[1][:])

                # ---- RMSNorm stats, split over DVE/ACT/GPSIMD so rsq is
                # ready before the q/k projection drains (the whole rope +
                # v-normalize chain gates on it) ----
                gen0 = emit_qkv_head(0)
                if True:
                    # ACT table preload: first ACT op loads the rsqrt set
                    # (which also holds Square/Copy) while ACT is idle
                    eps_t = cpool.tile([128, 1], f32, name="eps_t")
                    nc.gpsimd.memset(eps_t[:], EPS)
                    dum = npool.tile([128, 1], f32, name="dum")
                    nc.scalar.activation(
                        dum[:], xs_ch[0][:, 0:1],
                        mybir.ActivationFunctionType.Sqrt)

                    # 4 parallel accumulation chains (dk%4): squares on
                    # DVE (even c) / ACT (odd c); adds on DVE / GPSIMD
                    naccs = [None] * 4

                    def norm_dk(dk):
                        c = dk % 4
                        if naccs[c] is None:
                            acc = npool.tile([128, S], bf, name=f"nacc{c}")
                            if c % 2 == 1:
                                nc.scalar.square(acc[:], xs_v(dk))
                            else:
                                nc.vector.tensor_mul(acc[:], xs_v(dk), xs_v(dk))
                            naccs[c] = acc
                            return
                        sqt = sqpool.tile([128, S], bf, name="sqt")
                        if c % 2 == 1:
                            nc.scalar.square(sqt[:], xs_v(dk))
                        else:
                            nc.vector.tensor_mul(sqt[:], xs_v(dk), xs_v(dk))
                        # all adds on DVE: concurrent GPSIMD elementwise ops
                        # slow overlapping DVE ops ~3.5x (SBUF contention)
                        nc.vector.tensor_add(naccs[c][:], naccs[c][:], sqt[:])

                    def rsq_emit():
                        nacc_bf = naccs[0]
                        nc.vector.tensor_add(naccs[0][:], naccs[0][:], naccs[1][:])
                        nc.vector.tensor_add(naccs[2][:], naccs[2][:], naccs[3][:])
                        nc.vector.tensor_add(nacc_bf[:], naccs[0][:], naccs[2][:])
                        rsq = cpool.tile([128, S], f32, name="rsq")
                        for scc in range(2):
                            sl = slice(scc * 512, (scc + 1) * 512)
                            ps_ss = psS.tile([128, 512], f32, name="ps")
                            nc.tensor.matmul(ps_ss[:], ones_t[:], nacc_bf[:, sl],
                                             start=True, stop=True)
                            # sqrt(ss/D + eps) fused on ACT, then DVE rcp
                            ss_rt = sqpool.tile([128, 512], f32, name="sqt")
                            nc.scalar.activation(
                                ss_rt[:], ps_ss[:],
                                mybir.ActivationFunctionType.Sqrt,
                                bias=eps_t[:], scale=1.0 / D)
                            nc.vector.reciprocal_approx_fast(
                                rsq[:, sl], ss_rt[:])
                        box["rsq"] = rsq

                    box["norm_dk"] = norm_dk
                    box["rsq_emit"] = rsq_emit
                    # stage 1: q0+k0 projection + norm stats (129 resumes
                    # cover the 128 q/k matmuls plus the rsq/rope emission)
                    for _ in islice(gen0, 129):
                        pass

                # ======= heads 0-2: attention ⊗ chained QKV stream =======
                # One filler chain: v0 tail + transposes, then QKV heads 1-3.
                # Every junction (vN tail -> attention) interleaves into score
                # groups instead of serializing the PE on transpose copies.
                box["psT"] = (psS, "ps")

                def fill_chain():
                    yield from gen0
                    for hh in range(1, HL):
                        yield from emit_qkv_head(hh)

                fill = fill_chain()
                for h in range(3):
                    att_head(h, fill, psS, psO, last=False)
                    if _dbg and h == 0:
                        nc.sync.dma_start(vs0_dbg[:], vsb[0][:])
                        nc.sync.dma_start(at0_dbg[:], box["attnT_last"][:])
                    if _dbg and h == 1:
                        nc.sync.dma_start(vs1_dbg[:], vsb[1][:])
                for _ in fill:     # few leftover transposes of head 3
                    pass

            # =============== ATT_3 ⊗ output projection ===============
            with (
                tc.tile_pool(name="ag", bufs=1) as agpool,
                tc.tile_pool(name="yp", bufs=4) as yppool,
                tc.tile_pool(name="yout", bufs=2) as ypool,
            ):
                ag_sb = [None] * 3
                ag_sb3 = [None] * 2
                yparts = [None] * 4

                def outproj_gen():
                    for hh in range(3):
                        ag_sb[hh] = agpool.tile([128, 8 * S], bf, name="ag_sb",
                                                bufs=3)
                        nc.sync.dma_start(
                            ag_sb[hh][:].rearrange("p (r s) -> p r s", r=8),
                            ag_out[hh][:].rearrange("(r p) s -> p r s", p=128))
                    # hh-major: each head's block only gates on its own
                    # AllGather; partials accumulate in SBUF
                    for hh in range(3):
                        for oc8 in range(4):
                            ps = psp_pool.tile([128, S], f32, name="psp")
                            n = 0
                            for r in range(8):
                                for scc in range(2):
                                    nc.tensor.matmul(
                                        ps[:, scc * 512:(scc + 1) * 512],
                                        wos[hh][:, (r * 4 + oc8) * 128:
                                                (r * 4 + oc8 + 1) * 128],
                                        ag_sb[hh][:, r * S + scc * 512:
                                                  r * S + (scc + 1) * 512],
                                        start=(n < 2), stop=(n >= 14))
                                    n += 1
                                    yield
                            if hh == 0:
                                yp_t = yppool.tile([128, S], bf, name="ypart")
                                nc.scalar.copy(yp_t[:], ps[:])
                                yparts[oc8] = yp_t
                            else:
                                nc.vector.tensor_add(
                                    yparts[oc8][:], yparts[oc8][:], ps[:])

                def ag3_gather(sc):
                    ag_sb3[sc] = agpool.tile([128, 8 * 512], bf, name="ag_sb3",
                                             bufs=2)
                    nc.sync.dma_start(
                        ag_sb3[sc][:].rearrange("p (r s) -> p r s", r=8),
                        ag_out3[sc][:].rearrange("(r p) s -> p r s", p=128))

                box["ag3_gather"] = ag3_gather
                filler = outproj_gen()
                att_head(3, filler, psS, psO, last=True)
                for _ in filler:     # finish h0-2 out-proj parts (covers AG3)
                    pass

                # head-3 contributions per (oc8, s-half): the sc0 pass runs
                # while the sc1 AllGather is still in flight
                for scc in range(2):
                    for oc8 in range(4):
                        ps = psS.tile([128, 512], f32, name="ps")
                        for r in range(8):
                            nc.tensor.matmul(
                                ps[:],
                                wos[3][:, (r * 4 + oc8) * 128:
                                        (r * 4 + oc8 + 1) * 128],
                                ag_sb3[scc][:, r * 512:(r + 1) * 512],
                                start=(r == 0), stop=(r == 7))
                        ysb = ypool.tile([128, 512], f32, name="ysb")
                        nc.vector.tensor_add(
                            ysb[:], ps[:],
                            yparts[oc8][:, scc * 512:(scc + 1) * 512])
                        nc.sync.dma_start(
                            y[oc8 * 128:(oc8 + 1) * 128,
                              scc * 512:(scc + 1) * 512], ysb[:])
                if _dbg:
                    for i in range(3):
                        nc.sync.dma_start(agsb_dbg[i][:], ag_sb[i][:])

    nc.compile()
    return nc


def _host_prep(xs, cache_k, cache_v, norm_w, wq, wk, wv, wo):
    """Build the 8 per-core input maps (all layout work done on host)."""
    xs = np.asarray(xs, F32)
    cache_k = np.asarray(cache_k, F32)
    cache_v = np.asarray(cache_v, F32)
    norm_w = np.asarray(norm_w, F32)
    wq, wk, wv, wo = (np.asarray(w, F32) for w in (wq, wk, wv, wo))

    # xs^T tiled, chunk-major so each chunk DMA is contiguous: [16, 128, 2S]
    xs_t = np.ascontiguousarray(
        xs.T.reshape(NDK, 128, S).transpose(1, 0, 2).reshape(128, 16, 2 * S)
        .transpose(1, 0, 2)).astype(BF16)

    # RoPE tables (positions C..C+S-1), transposed [freq, s]
    half = HD // 2
    inv_freq = 1.0 / (THETA ** (np.arange(0, half, dtype=np.float64) * 2.0 / HD))
    pos = np.arange(S, dtype=np.float64) + C
    ang = np.outer(pos, inv_freq)          # [S, 64]
    cos1 = np.cos(ang).T.astype(F32)       # [64, S]
    sin1 = np.sin(ang).T.astype(F32)
    cosT = np.vstack([cos1, cos1]).astype(BF16)          # [128, S]
    sinT = np.vstack([-sin1, sin1]).astype(BF16)         # rotate-half signs

    # causal triangle for the diagonal 128x128 blocks: T[p, c] = 1 iff c >= p
    cc = np.arange(128)[None, :]
    pp = np.arange(128)[:, None]
    triW = (cc >= pp).astype(F32).astype(BF16)

    ones_d = np.ones((128, 128), F32).astype(BF16)
    id_d = np.eye(128, dtype=F32).astype(BF16)
    # half-swap permutation (symmetric): rotP.T @ x == [x2; x1]
    rotP = (((np.arange(128)[:, None] + 64) % 128)
            == np.arange(128)[None, :]).astype(F32).astype(BF16)

    sc_q = F32(1.0) / np.sqrt(F32(HD))

    in_maps = []
    for c in range(N_CORES):
        osl = slice(OC * c, OC * (c + 1))
        hsl = slice(HL * c, HL * (c + 1))
        # fold norm_w into wq/wk/wv; fold 1/sqrt(HD) into wq
        wq_c = (wq[osl] * norm_w[None, :]) * sc_q   # [512, 4096]
        wk_c = wk[osl] * norm_w[None, :]
        wv_c = wv[osl] * norm_w[None, :]

        def col_layout(w_c):
            # [HL, 128, NDK*128]: [h, p, dk*128 + j] = w_c[h*128+j, dk*128+p]
            m = w_c.reshape(HL, 128, NDK, 128)          # [h, j, dk, p]
            return np.ascontiguousarray(
                m.transpose(0, 3, 2, 1).reshape(HL, 128, NDK * 128)).astype(BF16)

        wq_col = col_layout(wq_c)
        wk_col = col_layout(wk_c)
        wv_col = col_layout(wv_c)

        # wo block for y^T: [h, p, (r*4 + oc8)*128 + j] =
        #   wo[OC*c + oc8*128 + j, (4r + h)*128 + p]
        wo_c = wo[osl]                                  # [512, 4096]
        m = wo_c.reshape(4, 128, 8, HL, 128)            # [oc8, j, r, h, p]
        wo_blk = np.ascontiguousarray(
            m.transpose(3, 4, 2, 0, 1).reshape(HL, 128, 8 * OC)).astype(BF16)

        # cache K^T per head: [h, p(hd), t]
        ck = np.ascontiguousarray(
            cache_k[:, hsl, :].transpose(1, 2, 0)).astype(BF16)   # [HL, 128, C]
        # cache V tiles: [h, p(t%128), ti*128 + hd]
        cv = np.ascontiguousarray(
            cache_v[:, hsl, :].reshape(NTC, 128, HL, HD)
            .transpose(2, 1, 0, 3).reshape(HL, 128, C)).astype(BF16)

        in_maps.append({
            "xs_t": xs_t, "wq_col": wq_col, "wk_col": wk_col, "wv_col": wv_col,
            "wo_blk": wo_blk, "ckt": ck, "cvr": cv,
            "cosT": cosT, "sinT": sinT, "triW": triW,
            "ones_d": ones_d, "id_d": id_d, "rotP_d": rotP,
        })
    return in_maps


_NC_CACHE = {}


def kernel(xs, cache_k, cache_v, norm_w, wq, wk, wv, wo, _trace=False):
    if "nc" not in _NC_CACHE:
        _NC_CACHE["nc"] = _build_nc()
    nc = _NC_CACHE["nc"]
    in_maps = _host_prep(xs, cache_k, cache_v, norm_w, wq, wk, wv, wo)
    res = bass_utils.run_bass_kernel_spmd(
        nc, in_maps, core_ids=list(range(N_CORES)), trace=_trace)
    out = np.concatenate(
        [res.results[c]["y"].T for c in range(N_CORES)], axis=1)
    out = np.ascontiguousarray(out)
    if _trace:
        kernel.last_exec_time_ns = res.exec_time_ns
        kernel.last_results = res
    return out

